# revision 5
# baseline (speedup 1.0000x reference)
"""Classical ray marcher (volume rendering) Bass kernel for 8 Trainium2 cores.

Problem: B=8, R=16384 rays, S=64 samples/ray, C=3 channels.
  dens   = softplus(densities)
  deltas = diff(depths) with last delta = 1e10
  alpha  = 1 - exp(-deltas*dens)
  t      = 1 - alpha + 1e-10
  trans  = cumprod(t) with leading 1          (per ray, over S)
  w      = alpha * trans[:-1]  ==  trans[s-1] - trans[s]   (up to 1e-10*trans)
  rgb    = sum_s w*colors ; depth = sum_s w*depths ; ft = trans[-1]

Sharding: core i handles batch i (embarrassingly parallel over rays).
On-chip layout: 128 rays on partitions, G rays x 64 samples on the free dim.
The per-ray cumprod runs as ONE hardware linear-recurrence scan per tile
(state = t*state + b), where b is zero except at each ray's first sample
(carries t[0]) - that resets the recurrence at ray boundaries.

Engine split per tile (load-balanced around the 1-elem/cycle fp32 DVE):
  GpSimd: delta sub, p=delta*dens, wd=w*depths, small memsets
  ACT:    exp/ln softplus, exp(-p), +eps, column fixups; store DMAs (HWDGE)
  DVE:    cumprod scan, w = c_prev - c, wc = w*colors, both reductions
  sync:   load DMAs (HWDGE)
"""

import numpy as np

import concourse.bacc as bacc
import concourse.mybir as mybir
from concourse import tile
from concourse.bass_utils import run_bass_kernel_spmd

F32 = mybir.dt.float32
AF = mybir.ActivationFunctionType
OP = mybir.AluOpType

B, R, S, C = 8, 16384, 64, 3
P = 128           # SBUF partitions (rays per partition-tile)
EPS = 1e-10

N_CORES = 8

_BUILT = {}


class _Bacc(bacc.Bacc):
    """Bacc with a pinned activation-table choice.

    Exp and Ln both live in the 'natural_log_exp_and_others' table, but the
    default table-choice pass assigns each activation the first table that
    contains its function, which ping-pongs Exp->exp_and_others /
    Ln->natural_log and inserts a ~1.3us ACT_TABLE_LOAD per switch. Strip
    Exp/Ln from every other table (list order and length unchanged, so
    act_func_set_ids stay canonical) so one table covers the whole kernel.
    """

    def insert_act_table_loads(self):
        from concourse.hw_specs import get_activation_tables
        import bass_rust as _br

        has_activation = any(
            isinstance(i, mybir.InstActivation)
            for b in self.main_func.blocks
            for i in b.instructions
        )
        if not has_activation:
            return
        keep = "natural_log_exp_and_others"
        strip = {AF.Exp, AF.Ln}
        tables = []
        for name, fns in get_activation_tables(self.m.arch).items():
            if name != keep:
                fns = set(fns) - strip
            tables.append((name, fns))
        _br.insert_act_table_loads(self, tables)


def _build(n_rays: int, g: int):
    """Build the single-core Bass module for n_rays rays, g rays/partition/tile."""
    nc = _Bacc("TRN2", target_bir_lowering=False, debug=False)

    w_free = g * S            # free width of S-sized tiles
    wc_free = g * S * C       # free width of color tiles
    rays_per_tile = P * g
    n_tiles = n_rays // rays_per_tile
    assert n_tiles * rays_per_tile == n_rays

    colors_d = nc.dram_tensor("colors", [n_rays, S * C], F32, kind="ExternalInput")
    dens_d = nc.dram_tensor("densities", [n_rays, S], F32, kind="ExternalInput")
    depths_d = nc.dram_tensor("depths", [n_rays, S], F32, kind="ExternalInput")

    w_out_d = nc.dram_tensor("weights", [n_rays, S], F32, kind="ExternalOutput")
    rgb_d = nc.dram_tensor("rgb", [n_rays, C], F32, kind="ExternalOutput")
    depth_d = nc.dram_tensor("depth_out", [n_rays], F32, kind="ExternalOutput")
    ft_d = nc.dram_tensor("ft", [n_rays], F32, kind="ExternalOutput")

    with tile.TileContext(nc) as tc:
        with (
            tc.tile_pool(name="const", bufs=1) as constp,
            tc.tile_pool(name="io", bufs=3) as io,
            tc.tile_pool(name="work", bufs=2) as work,
        ):
            # scan's additive input: all zeros except each ray's s=0 column,
            # which is rewritten per tile with that ray's t[0].
            b1 = constp.tile([P, w_free], F32)
            nc.vector.memset(b1[:], 0.0)

            for i in range(n_tiles):
                rays = slice(i * rays_per_tile, (i + 1) * rays_per_tile)

                col_t = io.tile([P, wc_free], F32, tag="col")
                dep_t = io.tile([P, w_free], F32, tag="dep")
                den_t = io.tile([P, w_free], F32, tag="den")
                nc.sync.dma_start(
                    col_t[:], colors_d.ap()[rays].rearrange("(p g) w -> p (g w)", p=P)
                )
                nc.sync.dma_start(
                    dep_t[:], depths_d.ap()[rays].rearrange("(p g) w -> p (g w)", p=P)
                )
                nc.sync.dma_start(
                    den_t[:], dens_d.ap()[rays].rearrange("(p g) w -> p (g w)", p=P)
                )

                dep3 = dep_t.rearrange("p (g s) -> p g s", g=g)

                # dens = softplus(raw) = Ln(Exp(raw)+1)   [ACT x2, one table]
                dens = work.tile([P, w_free], F32, tag="dens")
                nc.scalar.activation(dens[:], den_t[:], AF.Exp)
                nc.scalar.activation(dens[:], dens[:], AF.Ln, bias=1.0)
                dens3 = dens.rearrange("p (g s) -> p g s", g=g)

                # X: delta -> p -> e -> t, all in place
                x = work.tile([P, w_free], F32, tag="x")
                x3 = x.rearrange("p (g s) -> p g s", g=g)
                nc.gpsimd.tensor_tensor(
                    x3[:, :, 0:S - 1], dep3[:, :, 1:S], dep3[:, :, 0:S - 1],
                    OP.subtract,
                )
                nc.gpsimd.tensor_tensor(
                    x3[:, :, 0:S - 1], x3[:, :, 0:S - 1], dens3[:, :, 0:S - 1],
                    OP.mult,
                )
                nc.scalar.activation(
                    x3[:, :, 0:S - 1], x3[:, :, 0:S - 1], AF.Exp, scale=-1.0
                )
                nc.scalar.activation(
                    x3[:, :, 0:S - 1], x3[:, :, 0:S - 1], AF.Copy, bias=EPS
                )
                # t[63] = exp(-1e10*dens)+eps = eps exactly for any sane dens
                nc.gpsimd.memset(x3[:, :, S - 1:S], EPS)

                # segment-reset plumbing: b1[s=0] = t[0]; then t[0] := 0
                b13 = b1.rearrange("p (g s) -> p g s", g=g)
                nc.scalar.copy(b13[:, :, 0:1], x3[:, :, 0:1])
                nc.gpsimd.memset(x3[:, :, 0:1], 0.0)

                # c[s] = inclusive cumprod of t per ray    [DVE scan]
                c = work.tile([P, w_free], F32, tag="c")
                nc.vector.tensor_tensor_scan(
                    c[:], x[:], b1[:], 0.0, OP.mult, OP.add
                )
                c3 = c.rearrange("p (g s) -> p g s", g=g)

                # w[0] = 1 - c[0]; w[s] = c[s-1] - c[s]  (== alpha*trans, +-1e-10*c)
                w = work.tile([P, w_free], F32, tag="w")
                w3 = w.rearrange("p (g s) -> p g s", g=g)
                nc.scalar.activation(
                    w3[:, :, 0:1], c3[:, :, 0:1], AF.Copy, bias=1.0, scale=-1.0
                )
                nc.vector.tensor_tensor(
                    w3[:, :, 1:S], c3[:, :, 0:S - 1], c3[:, :, 1:S], OP.subtract
                )

                # wc = w (broadcast over C) * colors, written [g, c, s]-major
                # so the rgb reduction reads a contiguous inner axis [DVE]
                wc = work.tile([P, wc_free], F32, tag="wc")
                wcv = wc.rearrange("p (g c s) -> p g c s", g=g, c=C)
                col4 = col_t.rearrange("p (g s c) -> p g s c", g=g, s=S)
                wbc = w3.unsqueeze(3).broadcast_to([P, g, S, C])
                nc.vector.tensor_tensor(
                    wcv.transpose([0, 1, 3, 2]), col4[:], wbc, OP.mult
                )

                # rgb[g,c] = sum_s wc[g,c,s]               [DVE reduce, contig]
                rgb_t = work.tile([P, g * C], F32, tag="rgb_t")
                rgb3 = rgb_t.rearrange("p (g c) -> p g c", g=g)
                nc.vector.tensor_reduce(
                    rgb3[:], wcv[:], mybir.AxisListType.X, OP.add
                )

                # wd = w * depths (into X, dead after the scan)  [GpSimd]
                nc.gpsimd.tensor_tensor(x[:], w[:], dep_t[:], OP.mult)
                dep_o = work.tile([P, g], F32, tag="dep_o")
                nc.vector.tensor_reduce(
                    dep_o[:], x3[:], mybir.AxisListType.X, OP.add
                )

                # ft = c[63] (full cumprod incl. the 1e-10 last factor)
                ft_t = work.tile([P, g], F32, tag="ft_t")
                nc.scalar.copy(ft_t[:].unsqueeze(2), c3[:, :, S - 1:S])

                # stores on the ACT HWDGE ring (loads use the sync ring)
                nc.scalar.dma_start(
                    w_out_d.ap()[rays].rearrange("(p g) s -> p (g s)", p=P), w[:]
                )
                nc.scalar.dma_start(
                    rgb_d.ap()[rays].rearrange("(p g) c -> p (g c)", p=P), rgb_t[:]
                )
                nc.scalar.dma_start(
                    depth_d.ap()[rays].rearrange("(p g) -> p g", p=P), dep_o[:]
                )
                nc.scalar.dma_start(
                    ft_d.ap()[rays].rearrange("(p g) -> p g", p=P), ft_t[:]
                )

    nc.compile()
    return nc


def _get_nc(n_rays=R, g=16):
    key = (n_rays, g)
    if key not in _BUILT:
        _BUILT[key] = _build(n_rays, g)
    return _BUILT[key]


def _run(in_maps, n_rays=R, g=16, trace=False, **kw):
    nc = _get_nc(n_rays, g)
    return run_bass_kernel_spmd(nc, in_maps, list(range(len(in_maps))), trace=trace, **kw)


def kernel(colors, densities, depths):
    """Full-input entry point: colors [8,16384,64,3], densities/depths [8,16384,64,1].

    Returns (rgb_final [B,R,C], depth [B,R,1], weights [B,R,S,1], final_trans [B,R]).
    """
    colors = np.ascontiguousarray(colors, dtype=np.float32)
    densities = np.ascontiguousarray(densities, dtype=np.float32)
    depths = np.ascontiguousarray(depths, dtype=np.float32)

    in_maps = [
        {
            "colors": colors[i].reshape(R, S * C),
            "densities": densities[i].reshape(R, S),
            "depths": depths[i].reshape(R, S),
        }
        for i in range(B)
    ]
    res = _run(in_maps).results

    rgb = np.stack([res[i]["rgb"] for i in range(B)])                    # [B,R,C]
    depth = np.stack([res[i]["depth_out"] for i in range(B)])[..., None]  # [B,R,1]
    weights = np.stack([res[i]["weights"] for i in range(B)])[..., None]  # [B,R,S,1]
    ft = np.stack([res[i]["ft"] for i in range(B)])                      # [B,R]
    return rgb, depth, weights, ft
